# revision 33
# baseline (speedup 1.0000x reference)
"""Trainium2 Bass kernel for nn_Attention_48309792145474.

Multi-head attention (GQA 32q/8kv heads, head_dim 128, RoPE, causal) for
x:[2,2048,4096], tensor-parallel over heads across 8 NeuronCores.

v2 design (all-bf16 data path, fp32 PSUM accumulation):
  - per core c of 8: q-heads 4c..4c+3, kv-head c; wq/wk/wv column shards,
    wo row shard; host sums the 8 bf16 partial outputs.
  - host pre-transposes x -> xT (bf16) and permutes wq/wk columns so RoPE
    pairs land as [real(0:64); imag(64:128)] halves; 1/sqrt(hd) in wq.
  - phase Q: QKV GEMMs in bf16; RoPE on ACT(copies)+PE(swap)+DVE(muls);
    q/k/v stay SBUF-resident in bf16 (no DRAM roundtrip).
  - phase A: transposed-scores flash attention, bf16 operands:
      scoresT[tk,tq] = kT_tile^T @ qT  -> exp on ACT -> bf16 probs
      causal mask applied as a post-exp binary multiply (DVE 4x mode)
      row sums: bf16 acc (DVE 4x) + ones-matmul; reciprocal on DVE;
      broadcast via SBUF->SBUF DMA; normalize into attn_sb (bf16)
  - phase W: out chunk = attn^T @ wo rows, PSUM->SBUF bf16 on ACT,
    DMA bf16 partial [4096,4096] per core; host sums in fp32.
"""
import os
import sys

os.environ.setdefault("MYCRO_LOCAL_CACHE", "1")

for _p in ("/opt/trn_rl_repo",):
    if os.path.isdir(_p) and _p not in sys.path:
        sys.path.insert(0, _p)

import numpy as np  # noqa: E402
import ml_dtypes  # noqa: E402

import concourse.bass as bass  # noqa: E402
import concourse.mybir as mybir  # noqa: E402
from concourse import bacc, tile  # noqa: E402
from concourse.bass_utils import run_bass_kernel_spmd  # noqa: E402
from concourse.tile_rust import add_dep_helper  # noqa: E402
from contextlib import ExitStack  # noqa: E402

B, T, D = 2, 2048, 4096
H, HKV, HD = 32, 8, 128
BT = B * T
NCORE = 8
QH = H // NCORE          # 4 q-heads per core
CW = 512                 # phase-Q token-chunk width
TCH = BT // CW           # 8 chunks
KBLK = D // 128          # 32 contraction blocks

F32 = mybir.dt.float32
BF16 = mybir.dt.bfloat16
EXP = mybir.ActivationFunctionType.Exp
NPBF = ml_dtypes.bfloat16

LAST_EXEC_NS = None
_CACHE = {}


def _build_nc(repeats=1, phases="qaw"):
    nc = bacc.Bacc("TRN2", target_bir_lowering=False, debug=False, num_devices=NCORE)

    xt_d = nc.dram_tensor("xt", [TCH, D, CW], BF16, kind="ExternalInput").ap()
    # wq split into 4 load-chunks of 8 kb-blocks for startup pipelining
    wq_d = nc.dram_tensor("wqb", [KBLK, 128, 512], BF16, kind="ExternalInput").ap()
    wk_d = nc.dram_tensor("wkb", [KBLK, 128, 128], BF16, kind="ExternalInput").ap()
    wv_d = nc.dram_tensor("wvb", [KBLK, 128, 128], BF16, kind="ExternalInput").ap()
    wo_d = nc.dram_tensor("wob", [4, 128, 4096], BF16, kind="ExternalInput").ap()
    csa_d = nc.dram_tensor("csa", [128, BT], BF16, kind="ExternalInput").ap()
    csb_d = nc.dram_tensor("csb", [128, BT], BF16, kind="ExternalInput").ap()
    mk_d = nc.dram_tensor("maskt", [128, 2048], BF16, kind="ExternalInput").ap()
    id_d = nc.dram_tensor("ident", [128, 128], BF16, kind="ExternalInput").ap()
    sw_d = nc.dram_tensor("swp", [128, 128], BF16, kind="ExternalInput").ap()
    on_d = nc.dram_tensor("onesc", [128, 1], BF16, kind="ExternalInput").ap()
    out_d = nc.dram_tensor("out", [BT, D], BF16, kind="ExternalOutput").ap()

    with tile.TileContext(nc) as tc:
        with ExitStack() as S0:
            cons = S0.enter_context(tc.tile_pool(name="cons", bufs=1))
            mk_sb = cons.tile([128, 2048], BF16)
            ones = cons.tile([128, 1], BF16)
            nc.sync.dma_start(out=mk_sb, in_=mk_d)
            nc.sync.dma_start(out=ones, in_=on_d)

            for rep in range(repeats):
                inv_d = nc.dram_tensor(f"inv_i{rep}", [B * QH * 4, 512], F32).ap()
                # persistent per-rep SBUF: q/k/v (bf16), attn, wo
                P0 = tc.tile_pool(name=f"p0_{rep}", bufs=1)
                with P0 as p0:
                    qt_sb = p0.tile([128, QH * BT], BF16)   # per head [128, BT]
                    kt_sb = p0.tile([128, BT], BF16)
                    v_sb = p0.tile([128, 32 * 128], BF16)   # [tk-part, blk*128+d]

                    # ------------------------------------------------ phase Q
                    with ExitStack() as SQ:
                        wp = SQ.enter_context(tc.tile_pool(name="wp", bufs=1))
                        wq_sb = wp.tile([128, KBLK * 512], BF16)
                        wk_sb = wp.tile([128, KBLK * 128], BF16)
                        wv_sb = wp.tile([128, KBLK * 128], BF16)
                        csa_sb = wp.tile([128, BT], BF16)
                        csb_sb = wp.tile([128, BT], BF16)
                        xp = SQ.enter_context(tc.tile_pool(name="xp", bufs=8))
                        stg = SQ.enter_context(tc.tile_pool(name="stg", bufs=3))
                        psq = SQ.enter_context(tc.tile_pool(name="psq", bufs=1, space="PSUM"))

                        def load_xq(ch, only=None):
                            xqs = []
                            for q in range(4):
                                if only is not None and q not in only:
                                    xqs.append(None)
                                    continue
                                xq = xp.tile([128, 8 * CW], BF16, tag="xq", name=f"xq_{ch}_{q}")
                                nc.sync.dma_start(
                                    out=xq.rearrange("p (a m) -> p a m", a=8),
                                    in_=xt_d[ch, q * 1024:(q + 1) * 1024, :]
                                        .rearrange("(a p) m -> p a m", p=128))
                                xqs.append(xq)
                            return xqs

                        # startup-ordered weight loads: first halves/chunks
                        # land before chunk-0 x so kb 0 can start ~10us in
                        def wload(sb, dr, lo, hi, width):
                            nc.sync.dma_start(
                                out=sb.rearrange("p (a m) -> p a m", a=KBLK)[:, lo:hi, :],
                                in_=dr[lo:hi].rearrange("a p m -> p a m"))
                        # round 0 consumes wq immediately; wk/wv only at round 1
                        xq0a = load_xq(0, only={0})
                        wload(wq_sb, wq_d, 0, 8, 512)
                        xq0b = load_xq(0, only={1, 2, 3})
                        xq0 = [xq0a[0]] + xq0b[1:]
                        for wc in range(1, 4):
                            wload(wq_sb, wq_d, wc * 8, (wc + 1) * 8, 512)
                        wload(wk_sb, wk_d, 0, 32, 128)
                        wload(wv_sb, wv_d, 0, 32, 128)
                        nc.sync.dma_start(out=csa_sb, in_=csa_d)
                        nc.sync.dma_start(out=csb_sb, in_=csb_d)

                        for ch in range(TCH):
                            xqs = xq0 if ch == 0 else load_xq(ch)

                            # two rounds of 3 full banks each:
                            #   r0 = q0,q1,q2   r1 = q3,k,v
                            banks = []
                            for r, tags in ((0, ("bA", "bB", "bC")),
                                            (1, ("bD", "bE", "bF"))):
                                rb = [psq.tile([128, 512], F32, tag=t,
                                               name=f"{t}_{ch}") for t in tags]
                                banks.append(rb)
                                for kb in range(KBLK):
                                    rhs = xqs[kb // 8][:, (kb % 8) * CW:(kb % 8 + 1) * CW]
                                    st, sp = kb == 0, kb == KBLK - 1
                                    w0 = kb * 512
                                    cols = ([wq_sb[:, w0 + i * 128:w0 + (i + 1) * 128] for i in range(3)]
                                            if r == 0 else
                                            [wq_sb[:, w0 + 384:w0 + 512],
                                             wk_sb[:, kb * 128:(kb + 1) * 128],
                                             wv_sb[:, kb * 128:(kb + 1) * 128]])
                                    for bank, lhs in zip(rb, cols):
                                        nc.tensor.matmul(bank, lhs, rhs, start=st, stop=sp)

                            c0 = ch * CW
                            asl = csa_sb[:, c0:c0 + CW]
                            bsl = csb_sb[:, c0:c0 + CW]

                            def rope_out(ps, dst, name):
                                # z = [r; i] (psum -> sbuf bf16); zs = halves
                                # swapped via SBUF->SBUF DMA (partition move);
                                # out = z*[c;c] + zs*[-s;s]
                                z = stg.tile([128, CW], BF16, tag="z", name=f"z_{name}")
                                nc.scalar.copy(z, ps)
                                zs = stg.tile([128, CW], BF16, tag="zs", name=f"zs_{name}")
                                nc.sync.dma_start(out=zs[64:128, :], in_=z[0:64, :])
                                nc.sync.dma_start(out=zs[0:64, :], in_=z[64:128, :])
                                u = stg.tile([128, CW], BF16, tag="u", name=f"u_{name}")
                                w = stg.tile([128, CW], BF16, tag="w", name=f"w_{name}")
                                nc.vector.tensor_mul(u, z, asl)
                                nc.vector.tensor_mul(w, zs, bsl)
                                nc.vector.tensor_add(dst, u, w)

                            for hh in range(4):
                                ps = banks[hh // 3][hh % 3] if hh < 3 else banks[1][0]
                                rope_out(ps, qt_sb[:, hh * BT + c0:hh * BT + c0 + CW], f"{ch}_{hh}")
                            rope_out(banks[1][1], kt_sb[:, c0:c0 + CW], f"{ch}_k")

                            # v: copy vT psum -> sbuf bf16, DMA-transpose to [t, d]
                            vs = stg.tile([128, CW], BF16, tag="vs", name=f"vs_{ch}")
                            nc.scalar.copy(vs, banks[1][2])
                            for i in range(4):
                                nc.sync.dma_start_transpose(
                                    out=v_sb[:, (4 * ch + i) * 128:(4 * ch + i + 1) * 128],
                                    in_=vs[:, i * 128:(i + 1) * 128])

                    if phases == "q":
                        continue
                    # ---------------------------------------- phases A + W
                    with ExitStack() as SAW:
                        p1 = SAW.enter_context(tc.tile_pool(name=f"p1_{rep}", bufs=1))
                        attn_sb = p1.tile([128, QH * BT], BF16)
                        wo_sb = p1.tile([128, 4 * 4096], BF16)
                        nc.sync.dma_start(
                            out=wo_sb.rearrange("p (a m) -> p a m", a=4),
                            in_=wo_d.rearrange("a p m -> p a m"))

                        # ------------------------------------------ phase A
                        SA = SAW.enter_context(ExitStack())
                        prp = SA.enter_context(tc.tile_pool(name="prp", bufs=3))
                        acp = SA.enter_context(tc.tile_pool(name="acp", bufs=2))
                        ivp = SA.enter_context(tc.tile_pool(name="ivp", bufs=3))
                        pss_p = SA.enter_context(tc.tile_pool(name="pss", bufs=2, space="PSUM"))
                        pso_p = SA.enter_context(tc.tile_pool(name="pso", bufs=3, space="PSUM"))
                        psm_p = SA.enter_context(tc.tile_pool(name="psm", bufs=1, space="PSUM"))

                        def emit_fin(fin):
                            # sums -> reciprocal -> broadcast -> normalize;
                            # deferred one iteration so the chain latency
                            # hides under the next iteration's matmuls
                            pso, acc, it, col = fin
                            psm = psm_p.tile([1, 512], F32, tag="m", name=f"psm_{it}")
                            nc.tensor.matmul(psm, ones, acc[:, 0:512], start=True, stop=False)
                            nc.tensor.matmul(psm, ones, acc[:, 512:1024], start=False, stop=True)
                            inv_r = ivp.tile([1, 512], F32, tag="ivr", name=f"ivr_{it}")
                            nc.vector.reciprocal(inv_r, psm)
                            nc.sync.dma_start(out=inv_d[it:it + 1, :], in_=inv_r)
                            inv_b = ivp.tile([128, 512], F32, tag="ivb", name=f"ivb_{it}")
                            nc.sync.dma_start(
                                out=inv_b, in_=inv_d[it:it + 1, :].to_broadcast((128, 512)))
                            nc.vector.tensor_mul(attn_sb[:, col:col + 512], pso, inv_b)

                        pending_fin = None
                        for b in range(B):
                            for h in range(QH):
                                qof = h * BT + b * T
                                for jc in range(4):
                                    it = (b * QH + h) * 4 + jc
                                    npair = 2 * (jc + 1)
                                    pso = pso_p.tile([128, 512], F32, tag="o", name=f"pso_{it}")
                                    acc = acp.tile([128, 1024], BF16, tag="acc", name=f"acc_{it}")

                                    def emit_pv(m, probs):
                                        for half in range(2):
                                            tb = 2 * m + half
                                            nc.tensor.matmul(
                                                pso,
                                                v_sb[:, (b * 16 + tb) * 128:(b * 16 + tb + 1) * 128],
                                                probs[:, half * 512:(half + 1) * 512],
                                                start=(tb == 0), stop=(tb == npair * 2 - 1))

                                    pend = None
                                    for m in range(npair):
                                        pss = pss_p.tile([128, 1024], F32, tag="s", name=f"pss_{it}_{m}")
                                        for half in range(2):
                                            tb = 2 * m + half
                                            nc.tensor.matmul(
                                                pss[:, half * 512:(half + 1) * 512],
                                                kt_sb[:, b * T + tb * 128:b * T + (tb + 1) * 128],
                                                qt_sb[:, qof + jc * 512:qof + (jc + 1) * 512],
                                                start=True, stop=True)
                                        probs = prp.tile([128, 1024], BF16, tag="pr", name=f"pr_{it}_{m}")
                                        nc.scalar.activation(probs, pss, EXP)
                                        for half in range(2):
                                            o = 2 * m + half - 4 * jc
                                            if o >= 0:
                                                nc.vector.tensor_mul(
                                                    probs[:, half * 512:(half + 1) * 512],
                                                    probs[:, half * 512:(half + 1) * 512],
                                                    mk_sb[:, o * 512:(o + 1) * 512])
                                        if m == 0:
                                            nc.vector.tensor_copy(acc, probs)
                                        else:
                                            nc.vector.tensor_add(acc, acc, probs)
                                        # PV runs one m behind so exp latency
                                        # hides under the next scores matmuls
                                        if pend is not None:
                                            emit_pv(m - 1, pend)
                                        pend = probs
                                        if m == 0 and pending_fin is not None:
                                            emit_fin(pending_fin)
                                            pending_fin = None
                                    emit_pv(npair - 1, pend)
                                    pending_fin = (pso, acc, it, qof + jc * 512)
                        emit_fin(pending_fin)
                        SA.close()

                        if phases == "qa":
                            continue
                        # -------------------------------------- phase W
                        SW = SAW.enter_context(ExitStack())
                        orp = SW.enter_context(tc.tile_pool(name="orp", bufs=3))
                        psw_p = SW.enter_context(tc.tile_pool(name="psw", bufs=3, space="PSUM"))
                        for tb in range(BT // 128):
                            for ep in range(8):
                                psw = psw_p.tile([128, 512], F32, tag="w", name=f"psw_{tb}_{ep}")
                                for db in range(4):
                                    nc.tensor.matmul(
                                        psw,
                                        attn_sb[:, db * BT + tb * 128:db * BT + (tb + 1) * 128],
                                        wo_sb[:, db * 4096 + ep * 512:db * 4096 + (ep + 1) * 512],
                                        start=(db == 0), stop=(db == 3))
                                orow = orp.tile([128, 512], BF16, tag="or", name=f"or_{tb}_{ep}")
                                nc.scalar.copy(orow, psw)
                                nc.sync.dma_start(
                                    out=out_d[tb * 128:(tb + 1) * 128,
                                              ep * 512:(ep + 1) * 512],
                                    in_=orow)
    nc.finalize()
    return nc


def _host_prep(x, freqs, wq, wk, wv, wo, mask=None):
    if mask is None:
        ii = np.arange(T)[:, None]
        jj = np.arange(T)[None, :]
        mask = np.where(jj <= ii, np.float32(0.0), np.float32(-1e9))
    _kernel_mask = np.asarray(mask, dtype=np.float32)

    x = np.ascontiguousarray(np.asarray(x, dtype=np.float32).reshape(BT, D))
    xT = x.T.astype(NPBF)                                  # [D, BT] bf16
    xtc = np.ascontiguousarray(xT.reshape(D, TCH, CW).transpose(1, 0, 2))

    freqs = np.asarray(freqs, dtype=np.float32)
    cos = np.tile(np.cos(freqs).T, (1, B))                 # [64, BT]
    sin = np.tile(np.sin(freqs).T, (1, B))
    csa = np.concatenate([cos, cos], axis=0).astype(NPBF)
    csb = np.concatenate([-sin, sin], axis=0).astype(NPBF)
    swp = np.zeros((128, 128), np.float32)
    swp[np.arange(64), 64 + np.arange(64)] = 1.0
    swp[64 + np.arange(64), np.arange(64)] = 1.0

    perm = np.concatenate([np.arange(0, HD, 2), np.arange(1, HD, 2)])
    wq_p = (np.asarray(wq, dtype=np.float32).reshape(D, H, HD)[:, :, perm]
            .reshape(D, H * HD) / np.float32(np.sqrt(HD)))
    wk_p = np.asarray(wk, dtype=np.float32).reshape(D, HKV, HD)[:, :, perm].reshape(D, HKV * HD)
    wv = np.asarray(wv, dtype=np.float32)
    wo = np.asarray(wo, dtype=np.float32)

    # binary mask, transposed band layout:
    # maskt[:, o*512:(o+1)*512][i, j] = 1 if key (128*o+i) visible to query j
    maskt = np.concatenate(
        [(_kernel_mask[0:512, 128 * o:128 * o + 128] > -1.0).T.astype(np.float32)
         for o in range(4)],
        axis=1).astype(NPBF)                               # [128, 2048]
    ident = np.eye(128, dtype=np.float32).astype(NPBF)

    in_maps = []
    for c in range(NCORE):
        wq_c = np.ascontiguousarray(
            wq_p[:, c * 512:(c + 1) * 512].reshape(KBLK, 128, 512)).astype(NPBF)
        wk_c = np.ascontiguousarray(
            wk_p[:, c * HD:(c + 1) * HD].reshape(KBLK, 128, 128)).astype(NPBF)
        wv_c = np.ascontiguousarray(
            wv[:, c * HD:(c + 1) * HD].reshape(KBLK, 128, 128)).astype(NPBF)
        wo_c = np.ascontiguousarray(
            wo[c * 512:(c + 1) * 512, :].reshape(4, 128, 4096)).astype(NPBF)
        in_maps.append({
            "xt": xtc, "wqb": wq_c, "wkb": wk_c, "wvb": wv_c, "wob": wo_c,
            "csa": csa, "csb": csb, "maskt": maskt, "ident": ident,
            "swp": swp.astype(NPBF), "onesc": np.ones((128, 1), NPBF),
        })
    return in_maps


def kernel(x, freqs, mask, wq, wk, wv, wo, start_pos=0, **_kw):
    global LAST_EXEC_NS
    in_maps = _host_prep(x, freqs, wq, wk, wv, wo, mask=mask)
    if "nc" not in _CACHE:
        _CACHE["nc"] = _build_nc()
    nc = _CACHE["nc"]
    res = run_bass_kernel_spmd(nc, in_maps, core_ids=list(range(NCORE)), trace=False)
    LAST_EXEC_NS = getattr(res, "exec_time_ns", None)
    total = res.results[0]["out"].astype(np.float32)
    for c in range(1, NCORE):
        total = total + res.results[c]["out"].astype(np.float32)
    return total.reshape(B, T, D)


# revision 37
# speedup vs baseline: 1.0171x; 1.0171x over previous
"""Trainium2 Bass kernel for nn_Attention_48309792145474.

Multi-head attention (GQA 32q/8kv heads, head_dim 128, RoPE, causal) for
x:[2,2048,4096], tensor-parallel over heads across 8 NeuronCores.

v2 design (all-bf16 data path, fp32 PSUM accumulation):
  - per core c of 8: q-heads 4c..4c+3, kv-head c; wq/wk/wv column shards,
    wo row shard; host sums the 8 bf16 partial outputs.
  - host pre-transposes x -> xT (bf16) and permutes wq/wk columns so RoPE
    pairs land as [real(0:64); imag(64:128)] halves; 1/sqrt(hd) in wq.
  - phase Q: QKV GEMMs in bf16; RoPE on ACT(copies)+PE(swap)+DVE(muls);
    q/k/v stay SBUF-resident in bf16 (no DRAM roundtrip).
  - phase A: transposed-scores flash attention, bf16 operands:
      scoresT[tk,tq] = kT_tile^T @ qT  -> exp on ACT -> bf16 probs
      causal mask applied as a post-exp binary multiply (DVE 4x mode)
      row sums: bf16 acc (DVE 4x) + ones-matmul; reciprocal on DVE;
      broadcast via SBUF->SBUF DMA; normalize into attn_sb (bf16)
  - phase W: out chunk = attn^T @ wo rows, PSUM->SBUF bf16 on ACT,
    DMA bf16 partial [4096,4096] per core; host sums in fp32.
"""
import os
import sys

os.environ.setdefault("MYCRO_LOCAL_CACHE", "1")

for _p in ("/opt/trn_rl_repo",):
    if os.path.isdir(_p) and _p not in sys.path:
        sys.path.insert(0, _p)

import numpy as np  # noqa: E402
import ml_dtypes  # noqa: E402

import concourse.bass as bass  # noqa: E402
import concourse.mybir as mybir  # noqa: E402
from concourse import bacc, tile  # noqa: E402
from concourse.bass_utils import run_bass_kernel_spmd  # noqa: E402
from concourse.tile_rust import add_dep_helper  # noqa: E402
from contextlib import ExitStack  # noqa: E402

B, T, D = 2, 2048, 4096
H, HKV, HD = 32, 8, 128
BT = B * T
NCORE = 8
QH = H // NCORE          # 4 q-heads per core
CW = 512                 # phase-Q token-chunk width
TCH = BT // CW           # 8 chunks
KBLK = D // 128          # 32 contraction blocks

F32 = mybir.dt.float32
BF16 = mybir.dt.bfloat16
EXP = mybir.ActivationFunctionType.Exp
NPBF = ml_dtypes.bfloat16

LAST_EXEC_NS = None
_CACHE = {}


def _build_nc(repeats=1, phases="qaw"):
    nc = bacc.Bacc("TRN2", target_bir_lowering=False, debug=False, num_devices=NCORE)

    xt_d = nc.dram_tensor("xt", [TCH, D, CW], BF16, kind="ExternalInput").ap()
    # wq split into 4 load-chunks of 8 kb-blocks for startup pipelining
    wq_d = nc.dram_tensor("wqb", [KBLK, 128, 512], BF16, kind="ExternalInput").ap()
    wk_d = nc.dram_tensor("wkb", [KBLK, 128, 128], BF16, kind="ExternalInput").ap()
    wv_d = nc.dram_tensor("wvb", [KBLK, 128, 128], BF16, kind="ExternalInput").ap()
    wo_d = nc.dram_tensor("wob", [4, 128, 4096], BF16, kind="ExternalInput").ap()
    csa_d = nc.dram_tensor("csa", [128, BT], BF16, kind="ExternalInput").ap()
    csb_d = nc.dram_tensor("csb", [128, BT], BF16, kind="ExternalInput").ap()
    mk_d = nc.dram_tensor("maskt", [128, 2048], BF16, kind="ExternalInput").ap()
    id_d = nc.dram_tensor("ident", [128, 128], BF16, kind="ExternalInput").ap()
    sw_d = nc.dram_tensor("swp", [128, 128], BF16, kind="ExternalInput").ap()
    on_d = nc.dram_tensor("onesc", [128, 1], BF16, kind="ExternalInput").ap()
    out_d = nc.dram_tensor("out", [BT, D], BF16, kind="ExternalOutput").ap()

    with tile.TileContext(nc) as tc:
        with ExitStack() as S0:
            cons = S0.enter_context(tc.tile_pool(name="cons", bufs=1))
            mk_sb = cons.tile([128, 2048], BF16)
            ones = cons.tile([128, 1], BF16)
            nc.sync.dma_start(out=mk_sb, in_=mk_d)
            nc.sync.dma_start(out=ones, in_=on_d)

            for rep in range(repeats):
                inv_d = nc.dram_tensor(f"inv_i{rep}", [B * QH * 4, 512], F32).ap()
                # persistent per-rep SBUF: q/k/v (bf16), attn, wo
                P0 = tc.tile_pool(name=f"p0_{rep}", bufs=1)
                with P0 as p0:
                    qt_sb = p0.tile([128, QH * BT], BF16)   # per head [128, BT]
                    kt_sb = p0.tile([128, BT], BF16)
                    v_sb = p0.tile([128, 32 * 128], BF16)   # [tk-part, blk*128+d]

                    # ------------------------------------------------ phase Q
                    with ExitStack() as SQ:
                        wp = SQ.enter_context(tc.tile_pool(name="wp", bufs=1))
                        wq_sb = wp.tile([128, KBLK * 512], BF16)
                        wk_sb = wp.tile([128, KBLK * 128], BF16)
                        wv_sb = wp.tile([128, KBLK * 128], BF16)
                        csa_sb = wp.tile([128, BT], BF16)
                        csb_sb = wp.tile([128, BT], BF16)
                        xp = SQ.enter_context(tc.tile_pool(name="xp", bufs=8))
                        stg = SQ.enter_context(tc.tile_pool(name="stg", bufs=3))
                        psq = SQ.enter_context(tc.tile_pool(name="psq", bufs=1, space="PSUM"))

                        def load_xq(ch, only=None):
                            xqs = []
                            for q in range(4):
                                if only is not None and q not in only:
                                    xqs.append(None)
                                    continue
                                xq = xp.tile([128, 8 * CW], BF16, tag="xq", name=f"xq_{ch}_{q}")
                                nc.sync.dma_start(
                                    out=xq.rearrange("p (a m) -> p a m", a=8),
                                    in_=xt_d[ch, q * 1024:(q + 1) * 1024, :]
                                        .rearrange("(a p) m -> p a m", p=128))
                                xqs.append(xq)
                            return xqs

                        # startup-ordered weight loads: first halves/chunks
                        # land before chunk-0 x so kb 0 can start ~10us in
                        def wload(sb, dr, lo, hi, width):
                            nc.sync.dma_start(
                                out=sb.rearrange("p (a m) -> p a m", a=KBLK)[:, lo:hi, :],
                                in_=dr[lo:hi].rearrange("a p m -> p a m"))
                        # round 0 consumes wq immediately; wk/wv only at round 1
                        xq0a = load_xq(0, only={0})
                        wload(wq_sb, wq_d, 0, 8, 512)
                        xq0b = load_xq(0, only={1, 2, 3})
                        xq0 = [xq0a[0]] + xq0b[1:]
                        for wc in range(1, 4):
                            wload(wq_sb, wq_d, wc * 8, (wc + 1) * 8, 512)
                        wload(wk_sb, wk_d, 0, 32, 128)
                        wload(wv_sb, wv_d, 0, 32, 128)
                        nc.sync.dma_start(out=csa_sb, in_=csa_d)
                        nc.sync.dma_start(out=csb_sb, in_=csb_d)

                        for ch in range(TCH):
                            xqs = xq0 if ch == 0 else load_xq(ch)

                            # two rounds of 3 full banks each:
                            #   r0 = q0,q1,q2   r1 = q3,k,v
                            banks = []
                            for r, tags in ((0, ("bA", "bB", "bC")),
                                            (1, ("bD", "bE", "bF"))):
                                rb = [psq.tile([128, 512], F32, tag=t,
                                               name=f"{t}_{ch}") for t in tags]
                                banks.append(rb)
                                for kb in range(KBLK):
                                    rhs = xqs[kb // 8][:, (kb % 8) * CW:(kb % 8 + 1) * CW]
                                    st, sp = kb == 0, kb == KBLK - 1
                                    w0 = kb * 512
                                    cols = ([wq_sb[:, w0 + i * 128:w0 + (i + 1) * 128] for i in range(3)]
                                            if r == 0 else
                                            [wq_sb[:, w0 + 384:w0 + 512],
                                             wk_sb[:, kb * 128:(kb + 1) * 128],
                                             wv_sb[:, kb * 128:(kb + 1) * 128]])
                                    for bank, lhs in zip(rb, cols):
                                        nc.tensor.matmul(bank, lhs, rhs, start=st, stop=sp)

                            c0 = ch * CW
                            asl = csa_sb[:, c0:c0 + CW]
                            bsl = csb_sb[:, c0:c0 + CW]

                            def rope_out(ps, dst, name):
                                # z = [r; i] (psum -> sbuf bf16); zs = halves
                                # swapped via SBUF->SBUF DMA (partition move);
                                # out = z*[c;c] + zs*[-s;s]
                                z = stg.tile([128, CW], BF16, tag="z", name=f"z_{name}")
                                nc.scalar.copy(z, ps)
                                zs = stg.tile([128, CW], BF16, tag="zs", name=f"zs_{name}")
                                nc.sync.dma_start(out=zs[64:128, :], in_=z[0:64, :])
                                nc.sync.dma_start(out=zs[0:64, :], in_=z[64:128, :])
                                u = stg.tile([128, CW], BF16, tag="u", name=f"u_{name}")
                                w = stg.tile([128, CW], BF16, tag="w", name=f"w_{name}")
                                nc.vector.tensor_mul(u, z, asl)
                                nc.vector.tensor_mul(w, zs, bsl)
                                nc.vector.tensor_add(dst, u, w)

                            for hh in range(4):
                                ps = banks[hh // 3][hh % 3] if hh < 3 else banks[1][0]
                                rope_out(ps, qt_sb[:, hh * BT + c0:hh * BT + c0 + CW], f"{ch}_{hh}")
                            rope_out(banks[1][1], kt_sb[:, c0:c0 + CW], f"{ch}_k")

                            # v: copy vT psum -> sbuf bf16, DMA-transpose to [t, d]
                            vs = stg.tile([128, CW], BF16, tag="vs", name=f"vs_{ch}")
                            nc.scalar.copy(vs, banks[1][2])
                            for i in range(4):
                                nc.sync.dma_start_transpose(
                                    out=v_sb[:, (4 * ch + i) * 128:(4 * ch + i + 1) * 128],
                                    in_=vs[:, i * 128:(i + 1) * 128])

                    if phases == "q":
                        continue
                    # ---------------------------------------- phases A + W
                    with ExitStack() as SAW:
                        p1 = SAW.enter_context(tc.tile_pool(name=f"p1_{rep}", bufs=1))
                        attn_sb = p1.tile([128, QH * BT], BF16)
                        wo_sb = p1.tile([128, 4 * 4096], BF16)
                        nc.sync.dma_start(
                            out=wo_sb.rearrange("p (a m) -> p a m", a=4),
                            in_=wo_d.rearrange("a p m -> p a m"))

                        # ------------------------------------------ phase A
                        SA = SAW.enter_context(ExitStack())
                        prp = SA.enter_context(tc.tile_pool(name="prp", bufs=3))
                        acp = SA.enter_context(tc.tile_pool(name="acp", bufs=2))
                        ivp = SA.enter_context(tc.tile_pool(name="ivp", bufs=3))
                        pss_p = SA.enter_context(tc.tile_pool(name="pss", bufs=2, space="PSUM"))
                        pso_p = SA.enter_context(tc.tile_pool(name="pso", bufs=3, space="PSUM"))
                        psm_p = SA.enter_context(tc.tile_pool(name="psm", bufs=1, space="PSUM"))

                        def emit_fin(fin):
                            # sums -> reciprocal -> broadcast -> normalize;
                            # deferred one iteration so the chain latency
                            # hides under the next iteration's matmuls
                            pso, acc, it, col = fin
                            psm = psm_p.tile([1, 512], F32, tag="m", name=f"psm_{it}")
                            nc.tensor.matmul(psm, ones, acc[:, 0:512], start=True, stop=False)
                            nc.tensor.matmul(psm, ones, acc[:, 512:1024], start=False, stop=True)
                            inv_r = ivp.tile([1, 512], F32, tag="ivr", name=f"ivr_{it}")
                            nc.vector.reciprocal(inv_r, psm)
                            nc.sync.dma_start(out=inv_d[it:it + 1, :], in_=inv_r)
                            inv_b = ivp.tile([128, 512], F32, tag="ivb", name=f"ivb_{it}")
                            nc.sync.dma_start(
                                out=inv_b, in_=inv_d[it:it + 1, :].to_broadcast((128, 512)))
                            nc.vector.tensor_mul(attn_sb[:, col:col + 512], pso, inv_b)

                        pending_fin = None
                        for b in range(B):
                            for h in range(QH):
                                qof = h * BT + b * T
                                for jc in range(4):
                                    it = (b * QH + h) * 4 + jc
                                    npair = 2 * (jc + 1)
                                    pso = pso_p.tile([128, 512], F32, tag="o", name=f"pso_{it}")
                                    acc = acp.tile([128, 1024], BF16, tag="acc", name=f"acc_{it}")

                                    def emit_pv(m, probs):
                                        for half in range(2):
                                            tb = 2 * m + half
                                            nc.tensor.matmul(
                                                pso,
                                                v_sb[:, (b * 16 + tb) * 128:(b * 16 + tb + 1) * 128],
                                                probs[:, half * 512:(half + 1) * 512],
                                                start=(tb == 0), stop=(tb == npair * 2 - 1))

                                    pend = None
                                    for m in range(npair):
                                        pss = pss_p.tile([128, 1024], F32, tag="s", name=f"pss_{it}_{m}")
                                        for half in range(2):
                                            tb = 2 * m + half
                                            nc.tensor.matmul(
                                                pss[:, half * 512:(half + 1) * 512],
                                                kt_sb[:, b * T + tb * 128:b * T + (tb + 1) * 128],
                                                qt_sb[:, qof + jc * 512:qof + (jc + 1) * 512],
                                                start=True, stop=True)
                                        probs = prp.tile([128, 1024], BF16, tag="pr", name=f"pr_{it}_{m}")
                                        nc.scalar.activation(probs, pss, EXP)
                                        for half in range(2):
                                            o = 2 * m + half - 4 * jc
                                            if o >= 0:
                                                nc.vector.tensor_mul(
                                                    probs[:, half * 512:(half + 1) * 512],
                                                    probs[:, half * 512:(half + 1) * 512],
                                                    mk_sb[:, o * 512:(o + 1) * 512])
                                        if m == 0:
                                            nc.vector.tensor_copy(acc, probs)
                                        else:
                                            nc.vector.tensor_add(acc, acc, probs)
                                        # PV runs one m behind so exp latency
                                        # hides under the next scores matmuls
                                        if pend is not None:
                                            emit_pv(m - 1, pend)
                                        pend = probs
                                        if m == 0 and pending_fin is not None:
                                            emit_fin(pending_fin)
                                            pending_fin = None
                                    emit_pv(npair - 1, pend)
                                    pending_fin = (pso, acc, it, qof + jc * 512)
                        emit_fin(pending_fin)
                        SA.close()

                        if phases == "qa":
                            continue
                        # -------------------------------------- phase W
                        SW = SAW.enter_context(ExitStack())
                        orp = SW.enter_context(tc.tile_pool(name="orp", bufs=3))
                        psw_p = SW.enter_context(tc.tile_pool(name="psw", bufs=3, space="PSUM"))
                        for tb in range(BT // 128):
                            for ep in range(8):
                                psw = psw_p.tile([128, 512], F32, tag="w", name=f"psw_{tb}_{ep}")
                                for db in range(4):
                                    nc.tensor.matmul(
                                        psw,
                                        attn_sb[:, db * BT + tb * 128:db * BT + (tb + 1) * 128],
                                        wo_sb[:, db * 4096 + ep * 512:db * 4096 + (ep + 1) * 512],
                                        start=(db == 0), stop=(db == 3))
                                orow = orp.tile([128, 512], BF16, tag="or", name=f"or_{tb}_{ep}")
                                nc.scalar.copy(orow, psw)
                                nc.sync.dma_start(
                                    out=out_d[tb * 128:(tb + 1) * 128,
                                              ep * 512:(ep + 1) * 512],
                                    in_=orow)
    nc.finalize()
    return nc


def _host_prep(x, freqs, wq, wk, wv, wo, mask=None):
    if mask is None:
        ii = np.arange(T)[:, None]
        jj = np.arange(T)[None, :]
        mask = np.where(jj <= ii, np.float32(0.0), np.float32(-1e9))
    _kernel_mask = np.asarray(mask, dtype=np.float32)

    x = np.ascontiguousarray(np.asarray(x, dtype=np.float32).reshape(BT, D))
    xT = x.T.astype(NPBF)                                  # [D, BT] bf16
    xtc = np.ascontiguousarray(xT.reshape(D, TCH, CW).transpose(1, 0, 2))

    freqs = np.asarray(freqs, dtype=np.float32)
    cos = np.tile(np.cos(freqs).T, (1, B))                 # [64, BT]
    sin = np.tile(np.sin(freqs).T, (1, B))
    csa = np.concatenate([cos, cos], axis=0).astype(NPBF)
    csb = np.concatenate([-sin, sin], axis=0).astype(NPBF)
    swp = np.zeros((128, 128), np.float32)
    swp[np.arange(64), 64 + np.arange(64)] = 1.0
    swp[64 + np.arange(64), np.arange(64)] = 1.0

    perm = np.concatenate([np.arange(0, HD, 2), np.arange(1, HD, 2)])
    wq_p = (np.asarray(wq, dtype=np.float32).reshape(D, H, HD)[:, :, perm]
            .reshape(D, H * HD) / np.float32(np.sqrt(HD)))
    wk_p = np.asarray(wk, dtype=np.float32).reshape(D, HKV, HD)[:, :, perm].reshape(D, HKV * HD)
    wv = np.asarray(wv, dtype=np.float32)
    wo = np.asarray(wo, dtype=np.float32)

    # binary mask, transposed band layout:
    # maskt[:, o*512:(o+1)*512][i, j] = 1 if key (128*o+i) visible to query j
    maskt = np.concatenate(
        [(_kernel_mask[0:512, 128 * o:128 * o + 128] > -1.0).T.astype(np.float32)
         for o in range(4)],
        axis=1).astype(NPBF)                               # [128, 2048]
    ident = np.eye(128, dtype=np.float32).astype(NPBF)

    in_maps = []
    for c in range(NCORE):
        wq_c = np.ascontiguousarray(
            wq_p[:, c * 512:(c + 1) * 512].reshape(KBLK, 128, 512)).astype(NPBF)
        wk_c = np.ascontiguousarray(
            wk_p[:, c * HD:(c + 1) * HD].reshape(KBLK, 128, 128)).astype(NPBF)
        wv_c = np.ascontiguousarray(
            wv[:, c * HD:(c + 1) * HD].reshape(KBLK, 128, 128)).astype(NPBF)
        wo_c = np.ascontiguousarray(
            wo[c * 512:(c + 1) * 512, :].reshape(4, 128, 4096)).astype(NPBF)
        in_maps.append({
            "xt": xtc, "wqb": wq_c, "wkb": wk_c, "wvb": wv_c, "wob": wo_c,
            "csa": csa, "csb": csb, "maskt": maskt, "ident": ident,
            "swp": swp.astype(NPBF), "onesc": np.ones((128, 1), NPBF),
        })
    return in_maps


def kernel(x, freqs, mask, wq, wk, wv, wo, start_pos=0, **_kw):
    global LAST_EXEC_NS
    in_maps = _host_prep(x, freqs, wq, wk, wv, wo, mask=mask)
    if "nc" not in _CACHE:
        _CACHE["nc"] = _build_nc()
    nc = _CACHE["nc"]
    res = run_bass_kernel_spmd(nc, in_maps, core_ids=list(range(NCORE)), trace=False)
    LAST_EXEC_NS = getattr(res, "exec_time_ns", None)
    total = res.results[0]["out"].astype(np.float32)
    for c in range(1, NCORE):
        total = total + res.results[c]["out"].astype(np.float32)
    return total.reshape(B, T, D)


# revision 38
# speedup vs baseline: 1.0408x; 1.0234x over previous
"""Trainium2 Bass kernel for nn_Attention_48309792145474.

Multi-head attention (GQA 32q/8kv heads, head_dim 128, RoPE, causal) for
x:[2,2048,4096], tensor-parallel over heads across 8 NeuronCores.

v2 design (all-bf16 data path, fp32 PSUM accumulation):
  - per core c of 8: q-heads 4c..4c+3, kv-head c; wq/wk/wv column shards,
    wo row shard; host sums the 8 bf16 partial outputs.
  - host pre-transposes x -> xT (bf16) and permutes wq/wk columns so RoPE
    pairs land as [real(0:64); imag(64:128)] halves; 1/sqrt(hd) in wq.
  - phase Q: QKV GEMMs in bf16; RoPE on ACT(copies)+PE(swap)+DVE(muls);
    q/k/v stay SBUF-resident in bf16 (no DRAM roundtrip).
  - phase A: transposed-scores flash attention, bf16 operands:
      scoresT[tk,tq] = kT_tile^T @ qT  -> exp on ACT -> bf16 probs
      causal mask applied as a post-exp binary multiply (DVE 4x mode)
      row sums: bf16 acc (DVE 4x) + ones-matmul; reciprocal on DVE;
      broadcast via SBUF->SBUF DMA; normalize into attn_sb (bf16)
  - phase W: out chunk = attn^T @ wo rows, PSUM->SBUF bf16 on ACT,
    DMA bf16 partial [4096,4096] per core; host sums in fp32.
"""
import os
import sys

os.environ.setdefault("MYCRO_LOCAL_CACHE", "1")

for _p in ("/opt/trn_rl_repo",):
    if os.path.isdir(_p) and _p not in sys.path:
        sys.path.insert(0, _p)

import numpy as np  # noqa: E402
import ml_dtypes  # noqa: E402

import concourse.bass as bass  # noqa: E402
import concourse.mybir as mybir  # noqa: E402
from concourse import bacc, tile  # noqa: E402
from concourse.bass_utils import run_bass_kernel_spmd  # noqa: E402
from concourse.tile_rust import add_dep_helper  # noqa: E402
from contextlib import ExitStack  # noqa: E402

B, T, D = 2, 2048, 4096
H, HKV, HD = 32, 8, 128
BT = B * T
NCORE = 8
QH = H // NCORE          # 4 q-heads per core
CW = 512                 # phase-Q token-chunk width
TCH = BT // CW           # 8 chunks
KBLK = D // 128          # 32 contraction blocks

F32 = mybir.dt.float32
BF16 = mybir.dt.bfloat16
EXP = mybir.ActivationFunctionType.Exp
NPBF = ml_dtypes.bfloat16

LAST_EXEC_NS = None
_CACHE = {}


def _build_nc(repeats=1, phases="qaw"):
    nc = bacc.Bacc("TRN2", target_bir_lowering=False, debug=False, num_devices=NCORE)

    xt_d = nc.dram_tensor("xt", [TCH, D, CW], BF16, kind="ExternalInput").ap()
    # wq split into 4 load-chunks of 8 kb-blocks for startup pipelining
    wq_d = nc.dram_tensor("wqb", [KBLK, 128, 512], BF16, kind="ExternalInput").ap()
    wk_d = nc.dram_tensor("wkb", [KBLK, 128, 128], BF16, kind="ExternalInput").ap()
    wv_d = nc.dram_tensor("wvb", [KBLK, 128, 128], BF16, kind="ExternalInput").ap()
    wo_d = nc.dram_tensor("wob", [4, 128, 4096], BF16, kind="ExternalInput").ap()
    csa_d = nc.dram_tensor("csa", [128, BT], BF16, kind="ExternalInput").ap()
    csb_d = nc.dram_tensor("csb", [128, BT], BF16, kind="ExternalInput").ap()
    mk_d = nc.dram_tensor("maskt", [128, 2048], BF16, kind="ExternalInput").ap()
    id_d = nc.dram_tensor("ident", [128, 128], BF16, kind="ExternalInput").ap()
    sw_d = nc.dram_tensor("swp", [128, 128], BF16, kind="ExternalInput").ap()
    on_d = nc.dram_tensor("onesc", [128, 1], BF16, kind="ExternalInput").ap()
    out_d = nc.dram_tensor("out", [BT, D], BF16, kind="ExternalOutput").ap()

    with tile.TileContext(nc) as tc:
        with ExitStack() as S0:
            cons = S0.enter_context(tc.tile_pool(name="cons", bufs=1))
            mk_sb = cons.tile([128, 2048], BF16)
            ones = cons.tile([128, 1], BF16)
            nc.sync.dma_start(out=mk_sb, in_=mk_d)
            nc.sync.dma_start(out=ones, in_=on_d)

            for rep in range(repeats):
                inv_d = nc.dram_tensor(f"inv_i{rep}", [B * QH * 4, 512], F32).ap()
                # persistent per-rep SBUF: q/k/v (bf16), attn, wo
                P0 = tc.tile_pool(name=f"p0_{rep}", bufs=1)
                with P0 as p0:
                    qt_sb = p0.tile([128, QH * BT], BF16)   # per head [128, BT]
                    kt_sb = p0.tile([128, BT], BF16)
                    v_sb = p0.tile([128, 32 * 128], BF16)   # [tk-part, blk*128+d]

                    # ------------------------------------------------ phase Q
                    with ExitStack() as SQ:
                        wp = SQ.enter_context(tc.tile_pool(name="wp", bufs=1))
                        wq_sb = wp.tile([128, KBLK * 512], BF16)
                        wk_sb = wp.tile([128, KBLK * 128], BF16)
                        wv_sb = wp.tile([128, KBLK * 128], BF16)
                        csa_sb = wp.tile([128, BT], BF16)
                        csb_sb = wp.tile([128, BT], BF16)
                        xp = SQ.enter_context(tc.tile_pool(name="xp", bufs=8))
                        stg = SQ.enter_context(tc.tile_pool(name="stg", bufs=3))
                        psq = SQ.enter_context(tc.tile_pool(name="psq", bufs=1, space="PSUM"))

                        def load_xq(ch, only=None):
                            xqs = []
                            for q in range(4):
                                if only is not None and q not in only:
                                    xqs.append(None)
                                    continue
                                xq = xp.tile([128, 8 * CW], BF16, tag="xq", name=f"xq_{ch}_{q}")
                                nc.sync.dma_start(
                                    out=xq.rearrange("p (a m) -> p a m", a=8),
                                    in_=xt_d[ch, q * 1024:(q + 1) * 1024, :]
                                        .rearrange("(a p) m -> p a m", p=128))
                                xqs.append(xq)
                            return xqs

                        # startup-ordered weight loads: first halves/chunks
                        # land before chunk-0 x so kb 0 can start ~10us in
                        def wload(sb, dr, lo, hi, width):
                            nc.sync.dma_start(
                                out=sb.rearrange("p (a m) -> p a m", a=KBLK)[:, lo:hi, :],
                                in_=dr[lo:hi].rearrange("a p m -> p a m"))
                        # round 0 consumes wq immediately; wk/wv only at round 1
                        xq0a = load_xq(0, only={0})
                        wload(wq_sb, wq_d, 0, 8, 512)
                        xq0b = load_xq(0, only={1, 2, 3})
                        xq0 = [xq0a[0]] + xq0b[1:]
                        for wc in range(1, 4):
                            wload(wq_sb, wq_d, wc * 8, (wc + 1) * 8, 512)
                        wload(wk_sb, wk_d, 0, 32, 128)
                        wload(wv_sb, wv_d, 0, 32, 128)
                        nc.sync.dma_start(out=csa_sb, in_=csa_d)
                        nc.sync.dma_start(out=csb_sb, in_=csb_d)

                        for ch in range(TCH):
                            xqs = xq0 if ch == 0 else load_xq(ch)

                            # two rounds of 3 full banks each:
                            #   r0 = q0,q1,q2   r1 = q3,k,v
                            banks = []
                            for r, tags in ((0, ("bA", "bB", "bC")),
                                            (1, ("bD", "bE", "bF"))):
                                rb = [psq.tile([128, 512], F32, tag=t,
                                               name=f"{t}_{ch}") for t in tags]
                                banks.append(rb)
                                for kb in range(KBLK):
                                    rhs = xqs[kb // 8][:, (kb % 8) * CW:(kb % 8 + 1) * CW]
                                    st, sp = kb == 0, kb == KBLK - 1
                                    w0 = kb * 512
                                    cols = ([wq_sb[:, w0 + i * 128:w0 + (i + 1) * 128] for i in range(3)]
                                            if r == 0 else
                                            [wq_sb[:, w0 + 384:w0 + 512],
                                             wk_sb[:, kb * 128:(kb + 1) * 128],
                                             wv_sb[:, kb * 128:(kb + 1) * 128]])
                                    for bank, lhs in zip(rb, cols):
                                        nc.tensor.matmul(bank, lhs, rhs, start=st, stop=sp)

                            c0 = ch * CW
                            asl = csa_sb[:, c0:c0 + CW]
                            bsl = csb_sb[:, c0:c0 + CW]

                            def rope_out(ps, dst, name):
                                # z = [r; i] (psum -> sbuf bf16); zs = halves
                                # swapped via SBUF->SBUF DMA (partition move);
                                # out = z*[c;c] + zs*[-s;s]
                                z = stg.tile([128, CW], BF16, tag="z", name=f"z_{name}")
                                nc.scalar.copy(z, ps)
                                zs = stg.tile([128, CW], BF16, tag="zs", name=f"zs_{name}")
                                nc.sync.dma_start(out=zs[64:128, :], in_=z[0:64, :])
                                nc.sync.dma_start(out=zs[0:64, :], in_=z[64:128, :])
                                u = stg.tile([128, CW], BF16, tag="u", name=f"u_{name}")
                                w = stg.tile([128, CW], BF16, tag="w", name=f"w_{name}")
                                nc.vector.tensor_mul(u, z, asl)
                                nc.vector.tensor_mul(w, zs, bsl)
                                nc.vector.tensor_add(dst, u, w)

                            for hh in range(4):
                                ps = banks[hh // 3][hh % 3] if hh < 3 else banks[1][0]
                                rope_out(ps, qt_sb[:, hh * BT + c0:hh * BT + c0 + CW], f"{ch}_{hh}")
                            rope_out(banks[1][1], kt_sb[:, c0:c0 + CW], f"{ch}_k")

                            # v: copy vT psum -> sbuf bf16, DMA-transpose to [t, d]
                            vs = stg.tile([128, CW], BF16, tag="vs", name=f"vs_{ch}")
                            nc.scalar.copy(vs, banks[1][2])
                            for i in range(4):
                                nc.sync.dma_start_transpose(
                                    out=v_sb[:, (4 * ch + i) * 128:(4 * ch + i + 1) * 128],
                                    in_=vs[:, i * 128:(i + 1) * 128])

                    if phases == "q":
                        continue
                    # ---------------------------------------- phases A + W
                    with ExitStack() as SAW:
                        p1 = SAW.enter_context(tc.tile_pool(name=f"p1_{rep}", bufs=1))
                        attn_sb = p1.tile([128, QH * BT], BF16)
                        wo_sb = p1.tile([128, 4 * 4096], BF16)
                        nc.sync.dma_start(
                            out=wo_sb.rearrange("p (a m) -> p a m", a=4),
                            in_=wo_d.rearrange("a p m -> p a m"))

                        # ------------------------------------------ phase A
                        SA = SAW.enter_context(ExitStack())
                        prp = SA.enter_context(tc.tile_pool(name="prp", bufs=5))
                        acp = SA.enter_context(tc.tile_pool(name="acp", bufs=3))
                        ivp = SA.enter_context(tc.tile_pool(name="ivp", bufs=4))
                        pss_p = SA.enter_context(tc.tile_pool(name="pss", bufs=2, space="PSUM"))
                        pso_p = SA.enter_context(tc.tile_pool(name="pso", bufs=3, space="PSUM"))
                        psm_p = SA.enter_context(tc.tile_pool(name="psm", bufs=1, space="PSUM"))

                        def emit_fin(fin):
                            # sums -> reciprocal -> broadcast -> normalize;
                            # deferred one iteration so the chain latency
                            # hides under the next iteration's matmuls
                            pso, acc, it, col = fin
                            psm = psm_p.tile([1, 512], F32, tag="m", name=f"psm_{it}")
                            nc.tensor.matmul(psm, ones, acc[:, 0:512], start=True, stop=False)
                            nc.tensor.matmul(psm, ones, acc[:, 512:1024], start=False, stop=True)
                            inv_r = ivp.tile([1, 512], F32, tag="ivr", name=f"ivr_{it}")
                            nc.vector.reciprocal(inv_r, psm)
                            nc.sync.dma_start(out=inv_d[it:it + 1, :], in_=inv_r)
                            inv_b = ivp.tile([128, 512], F32, tag="ivb", name=f"ivb_{it}")
                            nc.sync.dma_start(
                                out=inv_b, in_=inv_d[it:it + 1, :].to_broadcast((128, 512)))
                            nc.vector.tensor_mul(attn_sb[:, col:col + 512], pso, inv_b)

                        pending_fin = None
                        for b in range(B):
                            for h in range(QH):
                                qof = h * BT + b * T
                                for jc in range(4):
                                    it = (b * QH + h) * 4 + jc
                                    npair = 2 * (jc + 1)
                                    pso = pso_p.tile([128, 512], F32, tag="o", name=f"pso_{it}")
                                    acc = acp.tile([128, 1024], BF16, tag="acc", name=f"acc_{it}")

                                    def emit_pv(m, probs):
                                        for half in range(2):
                                            tb = 2 * m + half
                                            nc.tensor.matmul(
                                                pso,
                                                v_sb[:, (b * 16 + tb) * 128:(b * 16 + tb + 1) * 128],
                                                probs[:, half * 512:(half + 1) * 512],
                                                start=(tb == 0), stop=(tb == npair * 2 - 1))

                                    pend = None
                                    for m in range(npair):
                                        pss = pss_p.tile([128, 1024], F32, tag="s", name=f"pss_{it}_{m}")
                                        for half in range(2):
                                            tb = 2 * m + half
                                            nc.tensor.matmul(
                                                pss[:, half * 512:(half + 1) * 512],
                                                kt_sb[:, b * T + tb * 128:b * T + (tb + 1) * 128],
                                                qt_sb[:, qof + jc * 512:qof + (jc + 1) * 512],
                                                start=True, stop=True)
                                        probs = prp.tile([128, 1024], BF16, tag="pr", name=f"pr_{it}_{m}")
                                        nc.scalar.activation(probs, pss, EXP)
                                        for half in range(2):
                                            o = 2 * m + half - 4 * jc
                                            if o >= 0:
                                                nc.vector.tensor_mul(
                                                    probs[:, half * 512:(half + 1) * 512],
                                                    probs[:, half * 512:(half + 1) * 512],
                                                    mk_sb[:, o * 512:(o + 1) * 512])
                                        if m == 0:
                                            nc.vector.tensor_copy(acc, probs)
                                        else:
                                            nc.vector.tensor_add(acc, acc, probs)
                                        # PV runs one m behind so exp latency
                                        # hides under the next scores matmuls
                                        if pend is not None:
                                            emit_pv(m - 1, pend)
                                        pend = probs
                                        if m == 0 and pending_fin is not None:
                                            emit_fin(pending_fin)
                                            pending_fin = None
                                    emit_pv(npair - 1, pend)
                                    pending_fin = (pso, acc, it, qof + jc * 512)
                        emit_fin(pending_fin)
                        SA.close()

                        if phases == "qa":
                            continue
                        # -------------------------------------- phase W
                        SW = SAW.enter_context(ExitStack())
                        orp = SW.enter_context(tc.tile_pool(name="orp", bufs=3))
                        psw_p = SW.enter_context(tc.tile_pool(name="psw", bufs=3, space="PSUM"))
                        for tb in range(BT // 128):
                            for ep in range(8):
                                psw = psw_p.tile([128, 512], F32, tag="w", name=f"psw_{tb}_{ep}")
                                for db in range(4):
                                    nc.tensor.matmul(
                                        psw,
                                        attn_sb[:, db * BT + tb * 128:db * BT + (tb + 1) * 128],
                                        wo_sb[:, db * 4096 + ep * 512:db * 4096 + (ep + 1) * 512],
                                        start=(db == 0), stop=(db == 3))
                                orow = orp.tile([128, 512], BF16, tag="or", name=f"or_{tb}_{ep}")
                                nc.scalar.copy(orow, psw)
                                nc.sync.dma_start(
                                    out=out_d[tb * 128:(tb + 1) * 128,
                                              ep * 512:(ep + 1) * 512],
                                    in_=orow)
    nc.finalize()
    return nc


def _host_prep(x, freqs, wq, wk, wv, wo, mask=None):
    if mask is None:
        ii = np.arange(T)[:, None]
        jj = np.arange(T)[None, :]
        mask = np.where(jj <= ii, np.float32(0.0), np.float32(-1e9))
    _kernel_mask = np.asarray(mask, dtype=np.float32)

    x = np.ascontiguousarray(np.asarray(x, dtype=np.float32).reshape(BT, D))
    xT = x.T.astype(NPBF)                                  # [D, BT] bf16
    xtc = np.ascontiguousarray(xT.reshape(D, TCH, CW).transpose(1, 0, 2))

    freqs = np.asarray(freqs, dtype=np.float32)
    cos = np.tile(np.cos(freqs).T, (1, B))                 # [64, BT]
    sin = np.tile(np.sin(freqs).T, (1, B))
    csa = np.concatenate([cos, cos], axis=0).astype(NPBF)
    csb = np.concatenate([-sin, sin], axis=0).astype(NPBF)
    swp = np.zeros((128, 128), np.float32)
    swp[np.arange(64), 64 + np.arange(64)] = 1.0
    swp[64 + np.arange(64), np.arange(64)] = 1.0

    perm = np.concatenate([np.arange(0, HD, 2), np.arange(1, HD, 2)])
    wq_p = (np.asarray(wq, dtype=np.float32).reshape(D, H, HD)[:, :, perm]
            .reshape(D, H * HD) / np.float32(np.sqrt(HD)))
    wk_p = np.asarray(wk, dtype=np.float32).reshape(D, HKV, HD)[:, :, perm].reshape(D, HKV * HD)
    wv = np.asarray(wv, dtype=np.float32)
    wo = np.asarray(wo, dtype=np.float32)

    # binary mask, transposed band layout:
    # maskt[:, o*512:(o+1)*512][i, j] = 1 if key (128*o+i) visible to query j
    maskt = np.concatenate(
        [(_kernel_mask[0:512, 128 * o:128 * o + 128] > -1.0).T.astype(np.float32)
         for o in range(4)],
        axis=1).astype(NPBF)                               # [128, 2048]
    ident = np.eye(128, dtype=np.float32).astype(NPBF)

    in_maps = []
    for c in range(NCORE):
        wq_c = np.ascontiguousarray(
            wq_p[:, c * 512:(c + 1) * 512].reshape(KBLK, 128, 512)).astype(NPBF)
        wk_c = np.ascontiguousarray(
            wk_p[:, c * HD:(c + 1) * HD].reshape(KBLK, 128, 128)).astype(NPBF)
        wv_c = np.ascontiguousarray(
            wv[:, c * HD:(c + 1) * HD].reshape(KBLK, 128, 128)).astype(NPBF)
        wo_c = np.ascontiguousarray(
            wo[c * 512:(c + 1) * 512, :].reshape(4, 128, 4096)).astype(NPBF)
        in_maps.append({
            "xt": xtc, "wqb": wq_c, "wkb": wk_c, "wvb": wv_c, "wob": wo_c,
            "csa": csa, "csb": csb, "maskt": maskt, "ident": ident,
            "swp": swp.astype(NPBF), "onesc": np.ones((128, 1), NPBF),
        })
    return in_maps


def kernel(x, freqs, mask, wq, wk, wv, wo, start_pos=0, **_kw):
    global LAST_EXEC_NS
    in_maps = _host_prep(x, freqs, wq, wk, wv, wo, mask=mask)
    if "nc" not in _CACHE:
        _CACHE["nc"] = _build_nc()
    nc = _CACHE["nc"]
    res = run_bass_kernel_spmd(nc, in_maps, core_ids=list(range(NCORE)), trace=False)
    LAST_EXEC_NS = getattr(res, "exec_time_ns", None)
    total = res.results[0]["out"].astype(np.float32)
    for c in range(1, NCORE):
        total = total + res.results[c]["out"].astype(np.float32)
    return total.reshape(B, T, D)
